# revision 51
# baseline (speedup 1.0000x reference)
"""Trainium2 Bass kernel: single-head causal attention, data-parallel x8.

Problem shapes (hardcoded): x [512, 256, 384] f32, Wq/Wk/Wv [384, 64] f32.
Output: [512, 256, 64] f32 = softmax(causal(q @ k^T / 8)) @ v per batch.

Sharding: pure data parallel on batch (64 batches/core); weights
replicated; no collectives. On-chip compute in fp16 with fp32 PSUM
accumulation (rel err ~4e-4 vs the 2e-2 gate).

v2 (this session) vs the session-1 baseline (128.1us re-measured):
  - Output staged in token-PAIR layout (partition p holds tokens 2p,
    2p+1 as adjacent columns): DRAM descriptor runs grow 256B -> 512B,
    clearing the <512B half-bandwidth DMA penalty on the 4.2MB output
    stream (measured dma-only mode: x 73.1us + out 15.7us). The pair
    layout falls out of the final attention@v matmuls via STRIDE-2
    stationary column selection from P' (6 matmuls of 65 cols; merging
    to 4 with skip_group_check measured 15us SLOWER - don't).
  - PSUM bank allocation is the hard constraint (8 banks of 2KB). The
    measured throughput cap was the sv tier (scores+v bank, occupied
    from v-projection until exp): svb=3 buffers + SINGLE o_ps buffer
    (its turnaround is short) bought 4us. xt_ps needs 2 (xtb=1 costs
    +38us: transposes stall behind the DVE copy). Pair-coupling sv into
    one 2-bank tile (pair_sv) costs +19us - single-buffering kills
    cross-pair overlap.
  - Engine assignment (measured by ablation, the cost model's engine
    budgets do NOT predict these margins): causal mask = one
    tensor_mul vs a 0/1 triangle on DVE per batch-PAIR on a pair p_sb
    tile (pool affine_select is 2-4us slower; GPSIMD cannot touch
    PSUM at all); q|k PSUM->SBUF copies pair-fused on ScalarE (moving
    kT to DVE costs +4us; unfusing +6us); v copy + recip + scale-mul
    on DVE; ones-column memset on GPSIMD.
  - x loaded per 4-batch SWDGE cast-DMA (fp32 HBM -> fp16 SBUF);
    chunk size 2/4/8 measured equivalent.

Measured (HW For_i slope, shared/contended terminal): full ~116-117us,
compute-only (x loaded once) ~116us, dma-only ~89us, x-load-only
~73us. The kernel is engine/latency-bound, not DMA-bound: halving PE
work changes nothing; per-instruction scheduling margins (~85-90ns of
exposed latency per cross-engine instruction per batch, ~7 such
instructions) dominate and fully explain the gap to the cost model's
~96us schedule. Session-1 baseline measured 128.1us under the same
conditions.

Also tried and rejected: DVE tensor_tensor divide (walrus rejects;
recip+mul stays), pre-written vx ones columns (correct but neutral),
group=16, deeper xnat staging, emitting vproj before qkproj (sim shows
a 288ns/pair DVE bubble waiting on vproj, but HW was too contended to
confirm a win). PSUM's 8 banks are the structural jail: xt 2 + qk 2 +
sv 3 + o 1; any deeper fusion that drops a tier to 1 buffer loses more
pipeline depth than instruction count saves.
"""

import os
from contextlib import ExitStack

import numpy as np

B, T, C, H = 512, 256, 384, 64
N_CORES = 8
B_LOCAL = B // N_CORES


def build_nc(b_local=B_LOCAL, group=8, repeat=None, mode="full", x_chunk=4,
             mask="dve", qk_fused=True, out4=False, pair_sv=False,
             svb=3, opsb=1, xtb=2, qkb=2, pair_mask=True, kt="act",
             split_sv=False, norm="recip", ones_pre=False, xnb=4):
    import concourse.mybir as mybir
    import concourse.tile as tile
    from concourse import bacc

    F32 = mybir.dt.float32
    F16 = mybir.dt.float16
    AF = mybir.ActivationFunctionType
    ALU = mybir.AluOpType

    assert b_local % group == 0 and group % 2 == 0
    n_groups = b_local // group

    nc = bacc.Bacc()
    x = nc.declare_dram_parameter("x", [b_local, T, C], F32, isOutput=False)
    wq = nc.declare_dram_parameter("Wq", [C, H], F32, isOutput=False)
    wk = nc.declare_dram_parameter("Wk", [C, H], F32, isOutput=False)
    wv = nc.declare_dram_parameter("Wv", [C, H], F32, isOutput=False)
    out = nc.declare_dram_parameter("out", [b_local, T, H], F32, isOutput=True)

    NT = T // 128   # 2 token chunks
    NCC = C // 128  # 3 contraction chunks
    H1 = H + 1      # v plus ones column
    SCALE = 1.0 / np.sqrt(H)

    with tile.TileContext(nc) as tc, ExitStack() as ctx:
        const = ctx.enter_context(tc.tile_pool(name="const", bufs=1))
        xnat_p = ctx.enter_context(
            tc.tile_pool(name="xnat", bufs=min(xnb, n_groups)))
        xt_ps_p = ctx.enter_context(tc.tile_pool(name="xt_ps", bufs=xtb, space="PSUM"))
        xt_p = ctx.enter_context(tc.tile_pool(name="xt", bufs=10))
        qk_ps_p = ctx.enter_context(tc.tile_pool(name="qk_ps", bufs=qkb, space="PSUM"))
        qk_p = ctx.enter_context(tc.tile_pool(name="qk", bufs=6))
        sv_ps_p = ctx.enter_context(tc.tile_pool(
            name="sv_ps", bufs=1 if pair_sv else svb, space="PSUM"))
        v_ps_p = (ctx.enter_context(tc.tile_pool(name="v_ps", bufs=1,
                                                 space="PSUM"))
                  if split_sv else None)
        p_p = ctx.enter_context(tc.tile_pool(name="p", bufs=8))
        vx_p = ctx.enter_context(tc.tile_pool(name="vx", bufs=6))
        o_ps_p = ctx.enter_context(tc.tile_pool(name="o_ps", bufs=opsb, space="PSUM"))
        r_p = ctx.enter_context(tc.tile_pool(name="r", bufs=8))
        ob_p = ctx.enter_context(tc.tile_pool(name="ob", bufs=4))

        # --- constants ---
        # Load fp32 weights via HWDGE, cast to fp16 on DVE.
        # wqk_sb: per cc chunk [wq_cc | wk_cc] -> one full-array stationary,
        # so q and k project together in 3 matmuls.
        wqk_sb = const.tile([128, NCC * 128], F16, tag="wqk")
        wv_sb = const.tile([128, NCC * H], F16, tag="wv")
        w_stage = const.tile([128, 3 * NCC * H], F32, tag="w_stage")
        for i, w in enumerate((wq, wk, wv)):
            nc.sync.dma_start(
                w_stage[:, i * NCC * H:(i + 1) * NCC * H],
                w.rearrange("(a p) h -> p a h", p=128))
        wqk_3d = wqk_sb[:].rearrange("p (a x) -> p a x", x=128)
        nc.vector.tensor_copy(
            wqk_3d[:, :, 0:H],
            w_stage[:, 0:NCC * H].rearrange("p (a h) -> p a h", h=H))
        nc.vector.tensor_copy(
            wqk_3d[:, :, H:128],
            w_stage[:, NCC * H:2 * NCC * H].rearrange("p (a h) -> p a h", h=H))
        nc.vector.tensor_copy(wv_sb[:], w_stage[:, 2 * NCC * H:3 * NCC * H])

        ones = const.tile([128, 128], F16, tag="ones")
        nc.vector.memset(ones[:], 1.0)
        # dummy exp: forces LoadActFuncSet during the initial DMA wait
        actwarm = const.tile([1, 1], F32, tag="actwarm")
        nc.scalar.activation(actwarm[:], ones[0:1, 0:1], AF.Exp, scale=1.0)

        # identity for TensorE transpose
        ident = const.tile([128, 128], F16, tag="ident")
        nc.gpsimd.affine_select(
            ident[:], ones[:], pattern=[[1, 128]], compare_op=ALU.is_equal,
            fill=0.0, base=0, channel_multiplier=-1,
        )
        tri = None
        if mask == "dve":
            # tri[p, j] = 1 if j >= p else 0
            tri = const.tile([128, 128], F16, tag="tri")
            nc.gpsimd.affine_select(
                tri[:], ones[:], pattern=[[1, 128]], compare_op=ALU.is_ge,
                fill=0.0, base=0, channel_multiplier=-1,
            )
        if ones_pre:
            # Pre-write the softmax-denominator ones columns into every vx
            # pool buffer ONCE; the in-loop v copies never touch column H,
            # so the ones persist across all pool rotations and For_i
            # iterations (drops the per-pair GPSIMD memset + its dep).
            for _ in range(6):  # must equal the vx pool's bufs
                vx_i = vx_p.tile([128, 2 * NT * H1], F16, tag="vx",
                                 name="vx_pre")
                nc.gpsimd.memset(
                    vx_i[:].rearrange("p (b n x) -> p b n x",
                                      n=NT, x=H1)[:, :, :, H:H1], 1.0)

        loop_cm = tc.For_i(0, repeat, 1) if repeat is not None else None
        if loop_cm is not None:
            loop_cm.__enter__()
        xnat_pre = None
        for g in range(n_groups):
            # fp32 -> fp16 cast during DMA (SWDGE); x natural layout,
            # columns [(bb*NT + n)*C + c].
            if mode == "compute" and g > 0:
                xnat = xnat_pre  # engine-rate probe: reuse group 0's x
            else:
                xnat = xnat_p.tile([128, group * NT * C], F16, tag="xnat")
                xnat_pre = xnat
                bb0 = 0
                while bb0 < group:
                    ch = min(x_chunk, group - bb0)
                    nc.gpsimd.dma_start(
                        xnat[:, bb0 * NT * C:(bb0 + ch) * NT * C],
                        x[g * group + bb0:g * group + bb0 + ch].rearrange(
                            "b (n p) c -> p b n c", p=128),
                    )
                    bb0 += ch
            if mode == "xonly":
                continue
            # ob staging: columns (b, j, h) with token t = 2p + j -> the
            # output DMA sees contiguous 512B (j, h) runs per (p, b).
            ob = ob_p.tile([128, group * NT * H], F32, tag="ob")
            if mode == "dma":
                nc.vector.memset(ob[:], 0.0)
            for pr in range(0 if mode == "dma" else group // 2):
                # qk_ps pair tile (one PSUM bank): batch half's projection
                # at cols half*T:(half+1)*T, partitions 0:64 = qT, 64:128 = kT
                qk_ps = qk_ps_p.tile([128, 2 * T], F32, tag="qk_ps")
                # qk pair SBUF tile [64, (half, q 0:T | k T:2T)]
                qk = qk_p.tile([H, 2 * 2 * T], F16, tag="qk")
                # vx pair tile [128, (half, n, 64 v + 1 one)]
                vx = vx_p.tile([128, 2 * NT * H1], F16, tag="vx")
                # "pe_half" diagnostic: only contraction chunk 0 feeds the
                # projections (wrong outputs; measures PE-rate sensitivity)
                ncc_eff = 1 if mode == "pe_half" else NCC
                # sv tile(s): each batch half's [384 scores | 128 v] is one
                # PSUM bank. pair_sv couples both halves in one 2-bank tile
                # (fewer instructions, but single-buffered: measured slower).
                SB = 512  # f32 cols per half (one bank)
                v_pair = None
                if pair_sv:
                    sv_pair = sv_ps_p.tile([128, 1024], F32, tag="sv_ps")
                    sv_of = [(sv_pair, 0), (sv_pair, SB)]
                elif split_sv:
                    # scores-only tiles: occupancy starts at the scores
                    # matmul instead of the much earlier v projection
                    sv_of = [
                        (sv_ps_p.tile([128, 384], F32, tag="sv_ps",
                                      name="sv_ps"), 0)
                        for _ in range(2)]
                    v_pair = v_ps_p.tile([128, 2 * NT * H], F32, tag="v_ps")
                else:
                    sv_of = [
                        (sv_ps_p.tile([128, 512], F32, tag="sv_ps",
                                      name="sv_ps"), 0)
                        for _ in range(2)]
                for half in range(2):
                    bb = pr * 2 + half
                    sv_t, so = sv_of[half]
                    # --- transpose x -> xT [c, t]; columns [cc*T + t] ---
                    xt_ps = xt_ps_p.tile([128, NCC * T], F16, tag="xt_ps")
                    for cc in range(ncc_eff):
                        for n in range(NT):
                            nc.tensor.transpose(
                                xt_ps[:, cc * T + n * 128:cc * T + (n + 1) * 128],
                                xnat[:, (bb * NT + n) * C + cc * 128:
                                     (bb * NT + n) * C + (cc + 1) * 128],
                                ident[:],
                            )
                    xt = xt_p.tile([128, NCC * T], F16, tag="xt")
                    nc.vector.tensor_copy(
                        xt[:, 0:ncc_eff * T], xt_ps[:, 0:ncc_eff * T])

                    # --- fused q|k projection into the pair tile ---
                    for cc in range(ncc_eff):
                        st = dict(start=(cc == 0), stop=(cc == ncc_eff - 1))
                        nc.tensor.matmul(
                            qk_ps[:, half * T:(half + 1) * T],
                            wqk_sb[:, cc * 128:(cc + 1) * 128],
                            xt[:, cc * T:(cc + 1) * T], **st)
                    # --- v projection (natural [t, h]) into this half's
                    # bank of the sv pair tile ---
                    for n in range(NT):
                        for cc in range(ncc_eff):
                            st = dict(start=(cc == 0), stop=(cc == ncc_eff - 1))
                            vdst = (
                                v_pair[:, (half * NT + n) * H:
                                       (half * NT + n + 1) * H]
                                if split_sv else
                                sv_t[:, so + 384 + n * H:so + 384 + (n + 1) * H])
                            nc.tensor.matmul(
                                vdst,
                                xt[:, cc * T + n * 128:cc * T + (n + 1) * 128],
                                wv_sb[:, cc * H:(cc + 1) * H], **st)

                # --- pair-fused PSUM->SBUF copies on ScalarE ---
                qk4 = qk[:].rearrange("p (b x) -> p b x", x=2 * T)
                if qk_fused:
                    nc.scalar.copy(
                        qk4[:, :, 0:T],
                        qk_ps[0:H, :].rearrange("p (b t) -> p b t", t=T))
                    kt_eng = nc.vector if kt == "dve" else nc.scalar
                    (kt_eng.tensor_copy if kt == "dve" else kt_eng.copy)(
                        qk4[:, :, T:2 * T],
                        qk_ps[H:128, :].rearrange("p (b t) -> p b t", t=T))
                else:
                    for hf in range(2):
                        nc.scalar.copy(qk4[:, hf, 0:T],
                                       qk_ps[0:H, hf * T:(hf + 1) * T])
                        nc.scalar.copy(qk4[:, hf, T:2 * T],
                                       qk_ps[H:128, hf * T:(hf + 1) * T])

                # vx: v copy on DVE (GPSIMD cannot touch PSUM);
                # pair-fused ones memset on GPSIMD (SBUF-only op)
                vx4 = vx[:].rearrange("p (b n x) -> p b n x", n=NT, x=H1)
                if pair_sv:
                    sv2 = sv_of[0][0][:].rearrange("p (b x) -> p b x", x=SB)
                    nc.vector.tensor_copy(
                        vx4[:, :, :, 0:H],
                        sv2[:, :, 384:512].rearrange(
                            "p b (n h) -> p b n h", h=H))
                elif split_sv:
                    nc.vector.tensor_copy(
                        vx4[:, :, :, 0:H],
                        v_pair[:].rearrange("p (b n h) -> p b n h",
                                            n=NT, h=H))
                else:
                    for half in range(2):
                        sv_t, so = sv_of[half]
                        nc.vector.tensor_copy(
                            vx4[:, half, :, 0:H],
                            sv_t[:, so + 384:so + 512].rearrange(
                                "p (n h) -> p n h", h=H))
                if not ones_pre:
                    nc.gpsimd.memset(vx4[:, :, :, H:H1], 1.0)

                # o_ps pair tile (one PSUM bank): batch half at cols
                # half*NT*H1; within a half, o_ps[p, (j, 0:65)], token
                # t = 2p + j, col 64 = softmax denominator.
                o_ps = o_ps_p.tile([128, 2 * NT * H1], F32, tag="o_ps")
                p_list = []
                p_pair = None
                if not pair_sv and pair_mask:
                    # pair SBUF tile: per-batch exp writes halves; ONE
                    # mask instruction covers both batches' diag blocks
                    p_pair = p_p.tile([128, 2 * (T + 128)], F16, tag="p_sb")
                for half in range(2):
                    sv_t, so = sv_of[half]
                    q0 = half * 2 * T           # qT cols for this batch
                    k0 = half * 2 * T + T       # kT cols
                    # --- scores (transposed): S'[s, t] = kT.T @ qT ---
                    # S'0: s in [0,128), all t (cols 0:256)
                    # S'1: s in [128,256), t in [128,256) (cols 256:384)
                    nc.tensor.matmul(
                        sv_t[:, so:so + T], qk[:, k0:k0 + 128],
                        qk[:, q0:q0 + T])
                    nc.tensor.matmul(
                        sv_t[:, so + T:so + T + 128],
                        qk[:, k0 + 128:k0 + T], qk[:, q0 + 128:q0 + T])

                    if not pair_sv:
                        # --- per-batch exp ---
                        if pair_mask:
                            p_sb = p_pair[:, half * (T + 128):
                                          (half + 1) * (T + 128)]
                        else:
                            p_sb = p_p.tile([128, T + 128], F16,
                                            tag="p_sb", name="p_sb")[:]
                        p_list.append(p_sb)
                        nc.scalar.activation(p_sb, sv_t[:, so:so + T + 128],
                                             AF.Exp, scale=SCALE)
                        if not pair_mask:
                            p3 = p_sb.rearrange(
                                "p (k x) -> p k x", x=128)[:, 0:3:2, :]
                            if mask == "pool":
                                nc.gpsimd.affine_select(
                                    p3, p3, pattern=[[0, 2], [1, 128]],
                                    compare_op=ALU.is_ge, fill=0.0, base=0,
                                    channel_multiplier=-1,
                                )
                            elif mask == "dve":
                                tri2 = tri[:].rearrange(
                                    "p (k x) -> p k x", k=1).broadcast_to(
                                    [128, 2, 128])
                                nc.vector.tensor_mul(p3, p3, tri2)
                if not pair_sv and pair_mask:
                    p3 = p_pair[:].rearrange(
                        "p (b k x) -> p b k x", x=128, k=3)[:, :, 0:3:2, :]
                    if mask == "pool":
                        nc.gpsimd.affine_select(
                            p3, p3, pattern=[[0, 2], [0, 2], [1, 128]],
                            compare_op=ALU.is_ge, fill=0.0, base=0,
                            channel_multiplier=-1,
                        )
                    elif mask == "dve":
                        tri2 = tri[:].rearrange(
                            "p (b k x) -> p b k x", b=1, k=1).broadcast_to(
                            [128, 2, 2, 128])
                        nc.vector.tensor_mul(p3, p3, tri2)

                if pair_sv:
                    # --- pair-fused exp (scale folded in) on ScalarE ---
                    p_pair = p_p.tile([128, 2 * (T + 128)], F16, tag="p_sb")
                    p_list = [p_pair[:, 0:T + 128], p_pair[:, T + 128:]]
                    p2 = p_pair[:].rearrange("p (b x) -> p b x", x=T + 128)
                    nc.scalar.activation(p2, sv2[:, :, 0:T + 128], AF.Exp,
                                         scale=SCALE)
                    # --- pair-fused causal mask over the diagonal blocks ---
                    p3 = p_pair[:].rearrange(
                        "p (b k x) -> p b k x", x=128, k=3)[:, :, 0:3:2, :]
                    if mask == "pool":
                        nc.gpsimd.affine_select(
                            p3, p3, pattern=[[0, 2], [0, 2], [1, 128]],
                            compare_op=ALU.is_ge, fill=0.0, base=0,
                            channel_multiplier=-1,
                        )
                    elif mask == "dve":
                        tri2 = tri[:].rearrange(
                            "p (b k x) -> p b k x", b=1, k=1).broadcast_to(
                            [128, 2, 2, 128])
                        nc.vector.tensor_mul(p3, p3, tri2)

                for half in range(2):
                    # --- out in pair layout via stride-2 stationaries ---
                    # t < 128 (parts 0:64): only s-chunk 0 contributes;
                    # t >= 128 (parts 64:128): both s-chunks accumulate.
                    # P' columns viewed as (t, parity): block0 = S'0 t<128,
                    # block1 = S'0 t>=128, block2 = S'1 (t>=128, s-chunk 1)
                    pj = p_list[half].rearrange(
                        "p (t two) -> p two t", two=2)
                    o0 = half * NT * H1
                    for j in range(2):
                        vxh = vx4[:, half]
                        if out4:
                            # mm1 covers all 128 out partitions (s-chunk 0);
                            # mm2 accumulates s-chunk 1 onto parts 64:128
                            # only. start/stop describe sim accumulation
                            # groups, not hardware: the split-partition
                            # continuation is HW-correct.
                            nc.tensor.matmul(
                                o_ps[:, o0 + j * H1:o0 + (j + 1) * H1],
                                pj[:, j, 0:128], vxh[:, 0, :],
                                start=True, stop=True, skip_group_check=True)
                            nc.tensor.matmul(
                                o_ps[H:128, o0 + j * H1:o0 + (j + 1) * H1],
                                pj[:, j, 128:192], vxh[:, 1, :],
                                start=False, stop=True, skip_group_check=True)
                        else:
                            nc.tensor.matmul(
                                o_ps[0:H, o0 + j * H1:o0 + (j + 1) * H1],
                                pj[:, j, 0:64], vxh[:, 0, :])
                            nc.tensor.matmul(
                                o_ps[H:128, o0 + j * H1:o0 + (j + 1) * H1],
                                pj[:, j, 64:128], vxh[:, 0, :],
                                start=True, stop=False)
                            nc.tensor.matmul(
                                o_ps[H:128, o0 + j * H1:o0 + (j + 1) * H1],
                                pj[:, j, 128:192], vxh[:, 1, :],
                                start=False, stop=True)

                # --- pair-fused normalization on DVE into staging ---
                o3 = o_ps[:].rearrange("p (q x) -> p q x", x=H1)  # q=(b,j)
                ob_v = ob[:, pr * 2 * NT * H:(pr + 1) * 2 * NT * H].rearrange(
                    "p (q h) -> p q h", h=H)
                if norm == "div":
                    # single fused instruction: o / denominator
                    nc.vector.tensor_tensor(
                        ob_v, o3[:, :, 0:H],
                        o3[:, :, H:H1].broadcast_to([128, 2 * NT, H]),
                        op=ALU.divide)
                else:
                    rec = r_p.tile([128, 2 * NT], F32, tag="rec")
                    nc.vector.reciprocal(rec[:], o3[:, :, H])
                    nc.vector.tensor_mul(
                        ob_v, o3[:, :, 0:H],
                        rec[:].rearrange("p (q o) -> p q o", o=1).broadcast_to(
                            [128, 2 * NT, H]))

            # two 256KB HWDGE DMAs per group; DRAM runs are 512B (j, h)
            # pairs thanks to the pair layout.
            half_g = group // 2
            for hh in range(2):
                nc.sync.dma_start(
                    out[g * group + hh * half_g:
                        g * group + (hh + 1) * half_g].rearrange(
                        "b (p j) h -> p b j h", j=NT),
                    ob[:, hh * half_g * NT * H:(hh + 1) * half_g * NT * H]
                    .rearrange("p (b j h) -> p b j h", j=NT, h=H))
        if loop_cm is not None:
            loop_cm.__exit__(None, None, None)

    nc.compile()
    return nc


_CACHED = {}


def _make_runner(nc):
    """Build a cached shard_map'd jit for an SPMD Bass program."""
    import jax
    from jax.experimental.shard_map import shard_map
    from jax.sharding import Mesh, NamedSharding, PartitionSpec

    import concourse.mybir as mybir
    from concourse.bass2jax import (
        _bass_exec_p, install_neuronx_cc_hook, partition_id_tensor)

    install_neuronx_cc_hook()

    partition_name = (
        nc.partition_id_tensor.name if nc.partition_id_tensor else None)
    in_names, out_names, out_avals, zero_outs = [], [], [], []
    for alloc in nc.m.functions[0].allocations:
        if not isinstance(alloc, mybir.MemoryLocationSet):
            continue
        name = alloc.memorylocations[0].name
        if alloc.kind == "ExternalInput":
            if name != partition_name:
                in_names.append(name)
        elif alloc.kind == "ExternalOutput":
            out_names.append(name)
            shape = tuple(alloc.tensor_shape)
            dtype = mybir.dt.np(alloc.dtype)
            out_avals.append(jax.core.ShapedArray(shape, dtype))
            zero_outs.append(np.zeros(shape, dtype))
    n_params = len(in_names)
    all_in = in_names + out_names
    if partition_name is not None:
        all_in = all_in + [partition_name]

    def _body(*args):
        operands = list(args)
        if partition_name is not None:
            operands.append(partition_id_tensor())
        outs = _bass_exec_p.bind(
            *operands,
            out_avals=tuple(out_avals),
            in_names=tuple(all_in),
            out_names=tuple(out_names),
            lowering_input_output_aliases=(),
            sim_require_finite=False,
            sim_require_nnan=False,
            nc=nc,
        )
        return tuple(outs)

    devices = jax.devices()[:N_CORES]
    mesh = Mesh(np.asarray(devices), ("core",))
    spec = PartitionSpec("core")
    n_args = n_params + len(out_names)
    sharded = jax.jit(
        shard_map(
            _body, mesh=mesh, in_specs=(spec,) * n_args,
            out_specs=(spec,) * len(out_names), check_rep=False,
        ),
        keep_unused=True,
    )
    sharding = NamedSharding(mesh, spec)
    return sharded, in_names, zero_outs, sharding


def _get_runner():
    if "runner" not in _CACHED:
        _CACHED["runner"] = _make_runner(build_nc())
    return _CACHED["runner"]


def _device_inputs(x, Wq, Wk, Wv, runner=None):
    import jax

    sharded, in_names, zero_outs, sharding = runner or _get_runner()
    x = np.ascontiguousarray(x, dtype=np.float32)
    assert x.shape == (B, T, C)
    host = {
        "x": x,
        "Wq": np.concatenate([np.asarray(Wq, np.float32)] * N_CORES, axis=0),
        "Wk": np.concatenate([np.asarray(Wk, np.float32)] * N_CORES, axis=0),
        "Wv": np.concatenate([np.asarray(Wv, np.float32)] * N_CORES, axis=0),
    }
    args = [host[n] for n in in_names]
    args += [
        np.zeros((N_CORES * z.shape[0], *z.shape[1:]), z.dtype) for z in zero_outs
    ]
    return [jax.device_put(a, sharding) for a in args]


def kernel(x, Wq, Wk, Wv):
    sharded, _, _, _ = _get_runner()
    args = _device_inputs(x, Wq, Wk, Wv)
    (out,) = sharded(*args)
    return np.asarray(out)


# revision 53
# speedup vs baseline: 1.0228x; 1.0228x over previous
"""Trainium2 Bass kernel: single-head causal attention, data-parallel x8.

Problem shapes (hardcoded): x [512, 256, 384] f32, Wq/Wk/Wv [384, 64] f32.
Output: [512, 256, 64] f32 = softmax(causal(q @ k^T / 8)) @ v per batch.

Sharding: pure data parallel on batch (64 batches/core); weights
replicated; no collectives. On-chip compute in fp16 with fp32 PSUM
accumulation (rel err ~4e-4 vs the 2e-2 gate).

v2 (this session) vs the session-1 baseline (128.1us re-measured):
  - Output staged in token-PAIR layout (partition p holds tokens 2p,
    2p+1 as adjacent columns): DRAM descriptor runs grow 256B -> 512B,
    clearing the <512B half-bandwidth DMA penalty on the 4.2MB output
    stream (measured dma-only mode: x 73.1us + out 15.7us). The pair
    layout falls out of the final attention@v matmuls via STRIDE-2
    stationary column selection from P' (6 matmuls of 65 cols; merging
    to 4 with skip_group_check measured 15us SLOWER - don't).
  - PSUM bank allocation is the hard constraint (8 banks of 2KB). The
    measured throughput cap was the sv tier (scores+v bank, occupied
    from v-projection until exp): svb=3 buffers + SINGLE o_ps buffer
    (its turnaround is short) bought 4us. xt_ps needs 2 (xtb=1 costs
    +38us: transposes stall behind the DVE copy). Pair-coupling sv into
    one 2-bank tile (pair_sv) costs +19us - single-buffering kills
    cross-pair overlap.
  - Engine assignment (measured by ablation, the cost model's engine
    budgets do NOT predict these margins): causal mask = one
    tensor_mul vs a 0/1 triangle on DVE per batch-PAIR on a pair p_sb
    tile (pool affine_select is 2-4us slower; GPSIMD cannot touch
    PSUM at all); q|k PSUM->SBUF copies pair-fused on ScalarE (moving
    kT to DVE costs +4us; unfusing +6us); v copy + recip + scale-mul
    on DVE; ones-column memset on GPSIMD.
  - x loaded per 4-batch SWDGE cast-DMA (fp32 HBM -> fp16 SBUF);
    chunk size 2/4/8 measured equivalent.

Measured (HW For_i slope, shared/contended terminal): full ~116-117us,
compute-only (x loaded once) ~116us, dma-only ~89us, x-load-only
~73us. The kernel is engine/latency-bound, not DMA-bound: halving PE
work changes nothing; per-instruction scheduling margins (~85-90ns of
exposed latency per cross-engine instruction per batch, ~7 such
instructions) dominate and fully explain the gap to the cost model's
~96us schedule. Session-1 baseline measured 128.1us under the same
conditions.

Also tried and rejected: DVE tensor_tensor divide (walrus rejects;
recip+mul stays), pre-written vx ones columns (correct but neutral),
group=16, deeper xnat staging, emitting vproj before qkproj (sim shows
a 288ns/pair DVE bubble waiting on vproj, but HW was too contended to
confirm a win). PSUM's 8 banks are the structural jail: xt 2 + qk 2 +
sv 3 + o 1; any deeper fusion that drops a tier to 1 buffer loses more
pipeline depth than instruction count saves.
"""

import os
from contextlib import ExitStack

import numpy as np

B, T, C, H = 512, 256, 384, 64
N_CORES = 8
B_LOCAL = B // N_CORES


def build_nc(b_local=B_LOCAL, group=8, repeat=None, mode="full", x_chunk=4,
             mask="dve", qk_fused=True, out4=False, pair_sv=False,
             svb=3, opsb=1, xtb=2, qkb=2, pair_mask=True, kt="act",
             split_sv=False, norm="recip", ones_pre=False, xnb=4,
             vfirst=False, xt_pair=False):
    import concourse.mybir as mybir
    import concourse.tile as tile
    from concourse import bacc

    F32 = mybir.dt.float32
    F16 = mybir.dt.float16
    AF = mybir.ActivationFunctionType
    ALU = mybir.AluOpType

    assert b_local % group == 0 and group % 2 == 0
    n_groups = b_local // group

    nc = bacc.Bacc()
    x = nc.declare_dram_parameter("x", [b_local, T, C], F32, isOutput=False)
    wq = nc.declare_dram_parameter("Wq", [C, H], F32, isOutput=False)
    wk = nc.declare_dram_parameter("Wk", [C, H], F32, isOutput=False)
    wv = nc.declare_dram_parameter("Wv", [C, H], F32, isOutput=False)
    out = nc.declare_dram_parameter("out", [b_local, T, H], F32, isOutput=True)

    NT = T // 128   # 2 token chunks
    NCC = C // 128  # 3 contraction chunks
    H1 = H + 1      # v plus ones column
    SCALE = 1.0 / np.sqrt(H)

    with tile.TileContext(nc) as tc, ExitStack() as ctx:
        const = ctx.enter_context(tc.tile_pool(name="const", bufs=1))
        xnat_p = ctx.enter_context(
            tc.tile_pool(name="xnat", bufs=min(xnb, n_groups)))
        xt_ps_p = ctx.enter_context(tc.tile_pool(name="xt_ps", bufs=xtb, space="PSUM"))
        xt_p = ctx.enter_context(tc.tile_pool(name="xt", bufs=10))
        qk_ps_p = ctx.enter_context(tc.tile_pool(name="qk_ps", bufs=qkb, space="PSUM"))
        qk_p = ctx.enter_context(tc.tile_pool(name="qk", bufs=6))
        sv_ps_p = ctx.enter_context(tc.tile_pool(
            name="sv_ps", bufs=1 if pair_sv else svb, space="PSUM"))
        v_ps_p = (ctx.enter_context(tc.tile_pool(name="v_ps", bufs=1,
                                                 space="PSUM"))
                  if split_sv else None)
        p_p = ctx.enter_context(tc.tile_pool(name="p", bufs=8))
        vx_p = ctx.enter_context(tc.tile_pool(name="vx", bufs=6))
        o_ps_p = ctx.enter_context(tc.tile_pool(name="o_ps", bufs=opsb, space="PSUM"))
        r_p = ctx.enter_context(tc.tile_pool(name="r", bufs=8))
        ob_p = ctx.enter_context(tc.tile_pool(name="ob", bufs=4))

        # --- constants ---
        # Load fp32 weights via HWDGE, cast to fp16 on DVE.
        # wqk_sb: per cc chunk [wq_cc | wk_cc] -> one full-array stationary,
        # so q and k project together in 3 matmuls.
        wqk_sb = const.tile([128, NCC * 128], F16, tag="wqk")
        wv_sb = const.tile([128, NCC * H], F16, tag="wv")
        w_stage = const.tile([128, 3 * NCC * H], F32, tag="w_stage")
        for i, w in enumerate((wq, wk, wv)):
            nc.sync.dma_start(
                w_stage[:, i * NCC * H:(i + 1) * NCC * H],
                w.rearrange("(a p) h -> p a h", p=128))
        wqk_3d = wqk_sb[:].rearrange("p (a x) -> p a x", x=128)
        nc.vector.tensor_copy(
            wqk_3d[:, :, 0:H],
            w_stage[:, 0:NCC * H].rearrange("p (a h) -> p a h", h=H))
        nc.vector.tensor_copy(
            wqk_3d[:, :, H:128],
            w_stage[:, NCC * H:2 * NCC * H].rearrange("p (a h) -> p a h", h=H))
        nc.vector.tensor_copy(wv_sb[:], w_stage[:, 2 * NCC * H:3 * NCC * H])

        ones = const.tile([128, 128], F16, tag="ones")
        nc.vector.memset(ones[:], 1.0)
        # dummy exp: forces LoadActFuncSet during the initial DMA wait
        actwarm = const.tile([1, 1], F32, tag="actwarm")
        nc.scalar.activation(actwarm[:], ones[0:1, 0:1], AF.Exp, scale=1.0)

        # identity for TensorE transpose
        ident = const.tile([128, 128], F16, tag="ident")
        nc.gpsimd.affine_select(
            ident[:], ones[:], pattern=[[1, 128]], compare_op=ALU.is_equal,
            fill=0.0, base=0, channel_multiplier=-1,
        )
        tri = None
        if mask == "dve":
            # tri[p, j] = 1 if j >= p else 0
            tri = const.tile([128, 128], F16, tag="tri")
            nc.gpsimd.affine_select(
                tri[:], ones[:], pattern=[[1, 128]], compare_op=ALU.is_ge,
                fill=0.0, base=0, channel_multiplier=-1,
            )
        if ones_pre:
            # Pre-write the softmax-denominator ones columns into every vx
            # pool buffer ONCE; the in-loop v copies never touch column H,
            # so the ones persist across all pool rotations and For_i
            # iterations (drops the per-pair GPSIMD memset + its dep).
            for _ in range(6):  # must equal the vx pool's bufs
                vx_i = vx_p.tile([128, 2 * NT * H1], F16, tag="vx",
                                 name="vx_pre")
                nc.gpsimd.memset(
                    vx_i[:].rearrange("p (b n x) -> p b n x",
                                      n=NT, x=H1)[:, :, :, H:H1], 1.0)

        loop_cm = tc.For_i(0, repeat, 1) if repeat is not None else None
        if loop_cm is not None:
            loop_cm.__enter__()
        xnat_pre = None
        for g in range(n_groups):
            # fp32 -> fp16 cast during DMA (SWDGE); x natural layout,
            # columns [(bb*NT + n)*C + c].
            if mode == "compute" and g > 0:
                xnat = xnat_pre  # engine-rate probe: reuse group 0's x
            else:
                xnat = xnat_p.tile([128, group * NT * C], F16, tag="xnat")
                xnat_pre = xnat
                bb0 = 0
                while bb0 < group:
                    ch = min(x_chunk, group - bb0)
                    nc.gpsimd.dma_start(
                        xnat[:, bb0 * NT * C:(bb0 + ch) * NT * C],
                        x[g * group + bb0:g * group + bb0 + ch].rearrange(
                            "b (n p) c -> p b n c", p=128),
                    )
                    bb0 += ch
            if mode == "xonly":
                continue
            # ob staging: columns (b, j, h) with token t = 2p + j -> the
            # output DMA sees contiguous 512B (j, h) runs per (p, b).
            ob = ob_p.tile([128, group * NT * H], F32, tag="ob")
            if mode == "dma":
                nc.vector.memset(ob[:], 0.0)
            for pr in range(0 if mode == "dma" else group // 2):
                # qk_ps pair tile (one PSUM bank): batch half's projection
                # at cols half*T:(half+1)*T, partitions 0:64 = qT, 64:128 = kT
                qk_ps = qk_ps_p.tile([128, 2 * T], F32, tag="qk_ps")
                # qk pair SBUF tile [64, (half, q 0:T | k T:2T)]
                qk = qk_p.tile([H, 2 * 2 * T], F16, tag="qk")
                # vx pair tile [128, (half, n, 64 v + 1 one)]
                vx = vx_p.tile([128, 2 * NT * H1], F16, tag="vx")
                # "pe_half" diagnostic: only contraction chunk 0 feeds the
                # projections (wrong outputs; measures PE-rate sensitivity)
                ncc_eff = 1 if mode == "pe_half" else NCC
                # sv tile(s): each batch half's [384 scores | 128 v] is one
                # PSUM bank. pair_sv couples both halves in one 2-bank tile
                # (fewer instructions, but single-buffered: measured slower).
                SB = 512  # f32 cols per half (one bank)
                v_pair = None
                if pair_sv:
                    sv_pair = sv_ps_p.tile([128, 1024], F32, tag="sv_ps")
                    sv_of = [(sv_pair, 0), (sv_pair, SB)]
                elif split_sv:
                    # scores-only tiles: occupancy starts at the scores
                    # matmul instead of the much earlier v projection
                    sv_of = [
                        (sv_ps_p.tile([128, 384], F32, tag="sv_ps",
                                      name="sv_ps"), 0)
                        for _ in range(2)]
                    v_pair = v_ps_p.tile([128, 2 * NT * H], F32, tag="v_ps")
                else:
                    sv_of = [
                        (sv_ps_p.tile([128, 512], F32, tag="sv_ps",
                                      name="sv_ps"), 0)
                        for _ in range(2)]
                # xt_pair: one 2-bank PSUM tile for both batches'
                # transposes + ONE pair-wide DVE copy (individual
                # transposes write 256B-aligned chunks, never crossing
                # the bank boundary at byte 2048)
                if xt_pair:
                    xt_ps2 = xt_ps_p.tile([128, 2 * NCC * T], F16,
                                          tag="xt_ps")
                    xt2 = xt_p.tile([128, 2 * NCC * T], F16, tag="xt")
                for half in range(2):
                    bb = pr * 2 + half
                    sv_t, so = sv_of[half]
                    # --- transpose x -> xT [c, t]; columns [cc*T + t] ---
                    if xt_pair:
                        xt_ps = xt_ps2[:, half * NCC * T:(half + 1) * NCC * T]
                        xt = xt2[:, half * NCC * T:(half + 1) * NCC * T]
                    else:
                        xt_ps = xt_ps_p.tile([128, NCC * T], F16,
                                             tag="xt_ps", name="xt_ps")[:]
                        xt = xt_p.tile([128, NCC * T], F16,
                                       tag="xt", name="xt")[:]
                    for cc in range(ncc_eff):
                        for n in range(NT):
                            nc.tensor.transpose(
                                xt_ps[:, cc * T + n * 128:cc * T + (n + 1) * 128],
                                xnat[:, (bb * NT + n) * C + cc * 128:
                                     (bb * NT + n) * C + (cc + 1) * 128],
                                ident[:],
                            )
                    if not xt_pair:
                        nc.vector.tensor_copy(
                            xt[:, 0:ncc_eff * T], xt_ps[:, 0:ncc_eff * T])

                    projs = []
                    projs.append(("qk", None))
                    projs.append(("v", None))
                    if vfirst:
                        projs.reverse()
                    if xt_pair and half == 0:
                        continue  # projections for both halves after copy
                    halves = [0, 1] if xt_pair else [half]
                    if xt_pair:
                        nc.vector.tensor_copy(xt2[:], xt_ps2[:])
                    for hh in halves:
                        if xt_pair:
                            xt = xt2[:, hh * NCC * T:(hh + 1) * NCC * T]
                            sv_t, so = sv_of[hh]
                        for kind, _ in projs:
                            if kind == "qk":
                                for cc in range(ncc_eff):
                                    st = dict(start=(cc == 0),
                                              stop=(cc == ncc_eff - 1))
                                    nc.tensor.matmul(
                                        qk_ps[:, hh * T:(hh + 1) * T],
                                        wqk_sb[:, cc * 128:(cc + 1) * 128],
                                        xt[:, cc * T:(cc + 1) * T], **st)
                            else:
                                for n in range(NT):
                                    for cc in range(ncc_eff):
                                        st = dict(start=(cc == 0),
                                                  stop=(cc == ncc_eff - 1))
                                        vdst = (
                                            v_pair[:, (hh * NT + n) * H:
                                                   (hh * NT + n + 1) * H]
                                            if split_sv else
                                            sv_t[:, so + 384 + n * H:
                                                 so + 384 + (n + 1) * H])
                                        nc.tensor.matmul(
                                            vdst,
                                            xt[:, cc * T + n * 128:
                                               cc * T + (n + 1) * 128],
                                            wv_sb[:, cc * H:(cc + 1) * H],
                                            **st)

                # --- pair-fused PSUM->SBUF copies on ScalarE ---
                qk4 = qk[:].rearrange("p (b x) -> p b x", x=2 * T)
                if qk_fused:
                    nc.scalar.copy(
                        qk4[:, :, 0:T],
                        qk_ps[0:H, :].rearrange("p (b t) -> p b t", t=T))
                    kt_eng = nc.vector if kt == "dve" else nc.scalar
                    (kt_eng.tensor_copy if kt == "dve" else kt_eng.copy)(
                        qk4[:, :, T:2 * T],
                        qk_ps[H:128, :].rearrange("p (b t) -> p b t", t=T))
                else:
                    for hf in range(2):
                        nc.scalar.copy(qk4[:, hf, 0:T],
                                       qk_ps[0:H, hf * T:(hf + 1) * T])
                        nc.scalar.copy(qk4[:, hf, T:2 * T],
                                       qk_ps[H:128, hf * T:(hf + 1) * T])

                # vx: v copy on DVE (GPSIMD cannot touch PSUM);
                # pair-fused ones memset on GPSIMD (SBUF-only op)
                vx4 = vx[:].rearrange("p (b n x) -> p b n x", n=NT, x=H1)
                if pair_sv:
                    sv2 = sv_of[0][0][:].rearrange("p (b x) -> p b x", x=SB)
                    nc.vector.tensor_copy(
                        vx4[:, :, :, 0:H],
                        sv2[:, :, 384:512].rearrange(
                            "p b (n h) -> p b n h", h=H))
                elif split_sv:
                    nc.vector.tensor_copy(
                        vx4[:, :, :, 0:H],
                        v_pair[:].rearrange("p (b n h) -> p b n h",
                                            n=NT, h=H))
                else:
                    for half in range(2):
                        sv_t, so = sv_of[half]
                        nc.vector.tensor_copy(
                            vx4[:, half, :, 0:H],
                            sv_t[:, so + 384:so + 512].rearrange(
                                "p (n h) -> p n h", h=H))
                if not ones_pre:
                    nc.gpsimd.memset(vx4[:, :, :, H:H1], 1.0)

                # o_ps pair tile (one PSUM bank): batch half at cols
                # half*NT*H1; within a half, o_ps[p, (j, 0:65)], token
                # t = 2p + j, col 64 = softmax denominator.
                o_ps = o_ps_p.tile([128, 2 * NT * H1], F32, tag="o_ps")
                p_list = []
                p_pair = None
                if not pair_sv and pair_mask:
                    # pair SBUF tile: per-batch exp writes halves; ONE
                    # mask instruction covers both batches' diag blocks
                    p_pair = p_p.tile([128, 2 * (T + 128)], F16, tag="p_sb")
                for half in range(2):
                    sv_t, so = sv_of[half]
                    q0 = half * 2 * T           # qT cols for this batch
                    k0 = half * 2 * T + T       # kT cols
                    # --- scores (transposed): S'[s, t] = kT.T @ qT ---
                    # S'0: s in [0,128), all t (cols 0:256)
                    # S'1: s in [128,256), t in [128,256) (cols 256:384)
                    nc.tensor.matmul(
                        sv_t[:, so:so + T], qk[:, k0:k0 + 128],
                        qk[:, q0:q0 + T])
                    nc.tensor.matmul(
                        sv_t[:, so + T:so + T + 128],
                        qk[:, k0 + 128:k0 + T], qk[:, q0 + 128:q0 + T])

                    if not pair_sv:
                        # --- per-batch exp ---
                        if pair_mask:
                            p_sb = p_pair[:, half * (T + 128):
                                          (half + 1) * (T + 128)]
                        else:
                            p_sb = p_p.tile([128, T + 128], F16,
                                            tag="p_sb", name="p_sb")[:]
                        p_list.append(p_sb)
                        nc.scalar.activation(p_sb, sv_t[:, so:so + T + 128],
                                             AF.Exp, scale=SCALE)
                        if not pair_mask:
                            p3 = p_sb.rearrange(
                                "p (k x) -> p k x", x=128)[:, 0:3:2, :]
                            if mask == "pool":
                                nc.gpsimd.affine_select(
                                    p3, p3, pattern=[[0, 2], [1, 128]],
                                    compare_op=ALU.is_ge, fill=0.0, base=0,
                                    channel_multiplier=-1,
                                )
                            elif mask == "dve":
                                tri2 = tri[:].rearrange(
                                    "p (k x) -> p k x", k=1).broadcast_to(
                                    [128, 2, 128])
                                nc.vector.tensor_mul(p3, p3, tri2)
                if not pair_sv and pair_mask:
                    p3 = p_pair[:].rearrange(
                        "p (b k x) -> p b k x", x=128, k=3)[:, :, 0:3:2, :]
                    if mask == "pool":
                        nc.gpsimd.affine_select(
                            p3, p3, pattern=[[0, 2], [0, 2], [1, 128]],
                            compare_op=ALU.is_ge, fill=0.0, base=0,
                            channel_multiplier=-1,
                        )
                    elif mask == "dve":
                        tri2 = tri[:].rearrange(
                            "p (b k x) -> p b k x", b=1, k=1).broadcast_to(
                            [128, 2, 2, 128])
                        nc.vector.tensor_mul(p3, p3, tri2)

                if pair_sv:
                    # --- pair-fused exp (scale folded in) on ScalarE ---
                    p_pair = p_p.tile([128, 2 * (T + 128)], F16, tag="p_sb")
                    p_list = [p_pair[:, 0:T + 128], p_pair[:, T + 128:]]
                    p2 = p_pair[:].rearrange("p (b x) -> p b x", x=T + 128)
                    nc.scalar.activation(p2, sv2[:, :, 0:T + 128], AF.Exp,
                                         scale=SCALE)
                    # --- pair-fused causal mask over the diagonal blocks ---
                    p3 = p_pair[:].rearrange(
                        "p (b k x) -> p b k x", x=128, k=3)[:, :, 0:3:2, :]
                    if mask == "pool":
                        nc.gpsimd.affine_select(
                            p3, p3, pattern=[[0, 2], [0, 2], [1, 128]],
                            compare_op=ALU.is_ge, fill=0.0, base=0,
                            channel_multiplier=-1,
                        )
                    elif mask == "dve":
                        tri2 = tri[:].rearrange(
                            "p (b k x) -> p b k x", b=1, k=1).broadcast_to(
                            [128, 2, 2, 128])
                        nc.vector.tensor_mul(p3, p3, tri2)

                for half in range(2):
                    # --- out in pair layout via stride-2 stationaries ---
                    # t < 128 (parts 0:64): only s-chunk 0 contributes;
                    # t >= 128 (parts 64:128): both s-chunks accumulate.
                    # P' columns viewed as (t, parity): block0 = S'0 t<128,
                    # block1 = S'0 t>=128, block2 = S'1 (t>=128, s-chunk 1)
                    pj = p_list[half].rearrange(
                        "p (t two) -> p two t", two=2)
                    o0 = half * NT * H1
                    for j in range(2):
                        vxh = vx4[:, half]
                        if out4:
                            # mm1 covers all 128 out partitions (s-chunk 0);
                            # mm2 accumulates s-chunk 1 onto parts 64:128
                            # only. start/stop describe sim accumulation
                            # groups, not hardware: the split-partition
                            # continuation is HW-correct.
                            nc.tensor.matmul(
                                o_ps[:, o0 + j * H1:o0 + (j + 1) * H1],
                                pj[:, j, 0:128], vxh[:, 0, :],
                                start=True, stop=True, skip_group_check=True)
                            nc.tensor.matmul(
                                o_ps[H:128, o0 + j * H1:o0 + (j + 1) * H1],
                                pj[:, j, 128:192], vxh[:, 1, :],
                                start=False, stop=True, skip_group_check=True)
                        else:
                            nc.tensor.matmul(
                                o_ps[0:H, o0 + j * H1:o0 + (j + 1) * H1],
                                pj[:, j, 0:64], vxh[:, 0, :])
                            nc.tensor.matmul(
                                o_ps[H:128, o0 + j * H1:o0 + (j + 1) * H1],
                                pj[:, j, 64:128], vxh[:, 0, :],
                                start=True, stop=False)
                            nc.tensor.matmul(
                                o_ps[H:128, o0 + j * H1:o0 + (j + 1) * H1],
                                pj[:, j, 128:192], vxh[:, 1, :],
                                start=False, stop=True)

                # --- pair-fused normalization on DVE into staging ---
                o3 = o_ps[:].rearrange("p (q x) -> p q x", x=H1)  # q=(b,j)
                ob_v = ob[:, pr * 2 * NT * H:(pr + 1) * 2 * NT * H].rearrange(
                    "p (q h) -> p q h", h=H)
                if norm == "div":
                    # single fused instruction: o / denominator
                    nc.vector.tensor_tensor(
                        ob_v, o3[:, :, 0:H],
                        o3[:, :, H:H1].broadcast_to([128, 2 * NT, H]),
                        op=ALU.divide)
                else:
                    rec = r_p.tile([128, 2 * NT], F32, tag="rec")
                    nc.vector.reciprocal(rec[:], o3[:, :, H])
                    nc.vector.tensor_mul(
                        ob_v, o3[:, :, 0:H],
                        rec[:].rearrange("p (q o) -> p q o", o=1).broadcast_to(
                            [128, 2 * NT, H]))

            # two 256KB HWDGE DMAs per group; DRAM runs are 512B (j, h)
            # pairs thanks to the pair layout.
            half_g = group // 2
            for hh in range(2):
                nc.sync.dma_start(
                    out[g * group + hh * half_g:
                        g * group + (hh + 1) * half_g].rearrange(
                        "b (p j) h -> p b j h", j=NT),
                    ob[:, hh * half_g * NT * H:(hh + 1) * half_g * NT * H]
                    .rearrange("p (b j h) -> p b j h", j=NT, h=H))
        if loop_cm is not None:
            loop_cm.__exit__(None, None, None)

    nc.compile()
    return nc


_CACHED = {}


def _make_runner(nc):
    """Build a cached shard_map'd jit for an SPMD Bass program."""
    import jax
    from jax.experimental.shard_map import shard_map
    from jax.sharding import Mesh, NamedSharding, PartitionSpec

    import concourse.mybir as mybir
    from concourse.bass2jax import (
        _bass_exec_p, install_neuronx_cc_hook, partition_id_tensor)

    install_neuronx_cc_hook()

    partition_name = (
        nc.partition_id_tensor.name if nc.partition_id_tensor else None)
    in_names, out_names, out_avals, zero_outs = [], [], [], []
    for alloc in nc.m.functions[0].allocations:
        if not isinstance(alloc, mybir.MemoryLocationSet):
            continue
        name = alloc.memorylocations[0].name
        if alloc.kind == "ExternalInput":
            if name != partition_name:
                in_names.append(name)
        elif alloc.kind == "ExternalOutput":
            out_names.append(name)
            shape = tuple(alloc.tensor_shape)
            dtype = mybir.dt.np(alloc.dtype)
            out_avals.append(jax.core.ShapedArray(shape, dtype))
            zero_outs.append(np.zeros(shape, dtype))
    n_params = len(in_names)
    all_in = in_names + out_names
    if partition_name is not None:
        all_in = all_in + [partition_name]

    def _body(*args):
        operands = list(args)
        if partition_name is not None:
            operands.append(partition_id_tensor())
        outs = _bass_exec_p.bind(
            *operands,
            out_avals=tuple(out_avals),
            in_names=tuple(all_in),
            out_names=tuple(out_names),
            lowering_input_output_aliases=(),
            sim_require_finite=False,
            sim_require_nnan=False,
            nc=nc,
        )
        return tuple(outs)

    devices = jax.devices()[:N_CORES]
    mesh = Mesh(np.asarray(devices), ("core",))
    spec = PartitionSpec("core")
    n_args = n_params + len(out_names)
    sharded = jax.jit(
        shard_map(
            _body, mesh=mesh, in_specs=(spec,) * n_args,
            out_specs=(spec,) * len(out_names), check_rep=False,
        ),
        keep_unused=True,
    )
    sharding = NamedSharding(mesh, spec)
    return sharded, in_names, zero_outs, sharding


def _get_runner():
    if "runner" not in _CACHED:
        _CACHED["runner"] = _make_runner(build_nc())
    return _CACHED["runner"]


def _device_inputs(x, Wq, Wk, Wv, runner=None):
    import jax

    sharded, in_names, zero_outs, sharding = runner or _get_runner()
    x = np.ascontiguousarray(x, dtype=np.float32)
    assert x.shape == (B, T, C)
    host = {
        "x": x,
        "Wq": np.concatenate([np.asarray(Wq, np.float32)] * N_CORES, axis=0),
        "Wk": np.concatenate([np.asarray(Wk, np.float32)] * N_CORES, axis=0),
        "Wv": np.concatenate([np.asarray(Wv, np.float32)] * N_CORES, axis=0),
    }
    args = [host[n] for n in in_names]
    args += [
        np.zeros((N_CORES * z.shape[0], *z.shape[1:]), z.dtype) for z in zero_outs
    ]
    return [jax.device_put(a, sharding) for a in args]


def kernel(x, Wq, Wk, Wv):
    sharded, _, _, _ = _get_runner()
    args = _device_inputs(x, Wq, Wk, Wv)
    (out,) = sharded(*args)
    return np.asarray(out)


# revision 54
# speedup vs baseline: 1.0294x; 1.0065x over previous
"""Trainium2 Bass kernel: single-head causal attention, data-parallel x8.

Problem shapes (hardcoded): x [512, 256, 384] f32, Wq/Wk/Wv [384, 64] f32.
Output: [512, 256, 64] f32 = softmax(causal(q @ k^T / 8)) @ v per batch.

Sharding: pure data parallel on batch (64 batches/core); weights
replicated; no collectives. On-chip compute in fp16 with fp32 PSUM
accumulation (rel err ~4e-4 vs the 2e-2 gate).

v2 (this session) vs the session-1 baseline (128.1us re-measured):
  - Output staged in token-PAIR layout (partition p holds tokens 2p,
    2p+1 as adjacent columns): DRAM descriptor runs grow 256B -> 512B,
    clearing the <512B half-bandwidth DMA penalty on the 4.2MB output
    stream (measured dma-only mode: x 73.1us + out 15.7us). The pair
    layout falls out of the final attention@v matmuls via STRIDE-2
    stationary column selection from P' (6 matmuls of 65 cols; merging
    to 4 with skip_group_check measured 15us SLOWER - don't).
  - PSUM bank allocation is the hard constraint (8 banks of 2KB). The
    measured throughput cap was the sv tier (scores+v bank, occupied
    from v-projection until exp): svb=3 buffers + SINGLE o_ps buffer
    (its turnaround is short) bought 4us. xt_ps needs 2 (xtb=1 costs
    +38us: transposes stall behind the DVE copy). Pair-coupling sv into
    one 2-bank tile (pair_sv) costs +19us - single-buffering kills
    cross-pair overlap.
  - Engine assignment (measured by ablation, the cost model's engine
    budgets do NOT predict these margins): causal mask = one
    tensor_mul vs a 0/1 triangle on DVE per batch-PAIR on a pair p_sb
    tile (pool affine_select is 2-4us slower; GPSIMD cannot touch
    PSUM at all); q|k PSUM->SBUF copies pair-fused on ScalarE (moving
    kT to DVE costs +4us; unfusing +6us); v copy + recip + scale-mul
    on DVE; ones-column memset on GPSIMD.
  - x loaded per 4-batch SWDGE cast-DMA (fp32 HBM -> fp16 SBUF);
    chunk size 2/4/8 measured equivalent.

Measured (HW For_i slope, shared/contended terminal): full 116.8-121.6us
across repeated runs of this exact config (median ~118; contention
spread +-3us), compute-only (x loaded once) ~116us, dma-only ~89us,
x-load-only ~73us. The kernel is engine/latency-bound, not DMA-bound:
halving PE work changes nothing; per-instruction scheduling margins
(~85-90ns of exposed latency per cross-engine instruction per batch,
~7 such instructions) dominate and fully explain the gap to the cost
model's ~96us schedule. Session-1 baseline measured 128.1us under the
same conditions.

Also tried and rejected: DVE tensor_tensor divide (walrus rejects;
recip+mul stays), pre-written vx ones columns (correct but neutral),
group=16, deeper xnat staging, emitting vproj before qkproj (sim shows
a 288ns/pair DVE bubble waiting on vproj, but HW was too contended to
confirm a win). PSUM's 8 banks are the structural jail: xt 2 + qk 2 +
sv 3 + o 1; any deeper fusion that drops a tier to 1 buffer loses more
pipeline depth than instruction count saves.
"""

import os
from contextlib import ExitStack

import numpy as np

B, T, C, H = 512, 256, 384, 64
N_CORES = 8
B_LOCAL = B // N_CORES


def build_nc(b_local=B_LOCAL, group=8, repeat=None, mode="full", x_chunk=4,
             mask="dve", qk_fused=True, out4=False, pair_sv=False,
             svb=3, opsb=1, xtb=2, qkb=2, pair_mask=True, kt="act",
             split_sv=False, norm="recip", ones_pre=False, xnb=4,
             vfirst=False, xt_pair=False):
    import concourse.mybir as mybir
    import concourse.tile as tile
    from concourse import bacc

    F32 = mybir.dt.float32
    F16 = mybir.dt.float16
    AF = mybir.ActivationFunctionType
    ALU = mybir.AluOpType

    assert b_local % group == 0 and group % 2 == 0
    n_groups = b_local // group

    nc = bacc.Bacc()
    x = nc.declare_dram_parameter("x", [b_local, T, C], F32, isOutput=False)
    wq = nc.declare_dram_parameter("Wq", [C, H], F32, isOutput=False)
    wk = nc.declare_dram_parameter("Wk", [C, H], F32, isOutput=False)
    wv = nc.declare_dram_parameter("Wv", [C, H], F32, isOutput=False)
    out = nc.declare_dram_parameter("out", [b_local, T, H], F32, isOutput=True)

    NT = T // 128   # 2 token chunks
    NCC = C // 128  # 3 contraction chunks
    H1 = H + 1      # v plus ones column
    SCALE = 1.0 / np.sqrt(H)

    with tile.TileContext(nc) as tc, ExitStack() as ctx:
        const = ctx.enter_context(tc.tile_pool(name="const", bufs=1))
        xnat_p = ctx.enter_context(
            tc.tile_pool(name="xnat", bufs=min(xnb, n_groups)))
        xt_ps_p = ctx.enter_context(tc.tile_pool(name="xt_ps", bufs=xtb, space="PSUM"))
        xt_p = ctx.enter_context(tc.tile_pool(name="xt", bufs=10))
        qk_ps_p = ctx.enter_context(tc.tile_pool(name="qk_ps", bufs=qkb, space="PSUM"))
        qk_p = ctx.enter_context(tc.tile_pool(name="qk", bufs=6))
        sv_ps_p = ctx.enter_context(tc.tile_pool(
            name="sv_ps", bufs=1 if pair_sv else svb, space="PSUM"))
        v_ps_p = (ctx.enter_context(tc.tile_pool(name="v_ps", bufs=1,
                                                 space="PSUM"))
                  if split_sv else None)
        p_p = ctx.enter_context(tc.tile_pool(name="p", bufs=8))
        vx_p = ctx.enter_context(tc.tile_pool(name="vx", bufs=6))
        o_ps_p = ctx.enter_context(tc.tile_pool(name="o_ps", bufs=opsb, space="PSUM"))
        r_p = ctx.enter_context(tc.tile_pool(name="r", bufs=8))
        ob_p = ctx.enter_context(tc.tile_pool(name="ob", bufs=4))

        # --- constants ---
        # Load fp32 weights via HWDGE, cast to fp16 on DVE.
        # wqk_sb: per cc chunk [wq_cc | wk_cc] -> one full-array stationary,
        # so q and k project together in 3 matmuls.
        wqk_sb = const.tile([128, NCC * 128], F16, tag="wqk")
        wv_sb = const.tile([128, NCC * H], F16, tag="wv")
        w_stage = const.tile([128, 3 * NCC * H], F32, tag="w_stage")
        for i, w in enumerate((wq, wk, wv)):
            nc.sync.dma_start(
                w_stage[:, i * NCC * H:(i + 1) * NCC * H],
                w.rearrange("(a p) h -> p a h", p=128))
        wqk_3d = wqk_sb[:].rearrange("p (a x) -> p a x", x=128)
        nc.vector.tensor_copy(
            wqk_3d[:, :, 0:H],
            w_stage[:, 0:NCC * H].rearrange("p (a h) -> p a h", h=H))
        nc.vector.tensor_copy(
            wqk_3d[:, :, H:128],
            w_stage[:, NCC * H:2 * NCC * H].rearrange("p (a h) -> p a h", h=H))
        nc.vector.tensor_copy(wv_sb[:], w_stage[:, 2 * NCC * H:3 * NCC * H])

        ones = const.tile([128, 128], F16, tag="ones")
        nc.vector.memset(ones[:], 1.0)
        # dummy exp: forces LoadActFuncSet during the initial DMA wait
        actwarm = const.tile([1, 1], F32, tag="actwarm")
        nc.scalar.activation(actwarm[:], ones[0:1, 0:1], AF.Exp, scale=1.0)

        # identity for TensorE transpose
        ident = const.tile([128, 128], F16, tag="ident")
        nc.gpsimd.affine_select(
            ident[:], ones[:], pattern=[[1, 128]], compare_op=ALU.is_equal,
            fill=0.0, base=0, channel_multiplier=-1,
        )
        tri = None
        if mask == "dve":
            # tri[p, j] = 1 if j >= p else 0
            tri = const.tile([128, 128], F16, tag="tri")
            nc.gpsimd.affine_select(
                tri[:], ones[:], pattern=[[1, 128]], compare_op=ALU.is_ge,
                fill=0.0, base=0, channel_multiplier=-1,
            )
        if ones_pre:
            # Pre-write the softmax-denominator ones columns into every vx
            # pool buffer ONCE; the in-loop v copies never touch column H,
            # so the ones persist across all pool rotations and For_i
            # iterations (drops the per-pair GPSIMD memset + its dep).
            for _ in range(6):  # must equal the vx pool's bufs
                vx_i = vx_p.tile([128, 2 * NT * H1], F16, tag="vx",
                                 name="vx_pre")
                nc.gpsimd.memset(
                    vx_i[:].rearrange("p (b n x) -> p b n x",
                                      n=NT, x=H1)[:, :, :, H:H1], 1.0)

        loop_cm = tc.For_i(0, repeat, 1) if repeat is not None else None
        if loop_cm is not None:
            loop_cm.__enter__()
        xnat_pre = None
        for g in range(n_groups):
            # fp32 -> fp16 cast during DMA (SWDGE); x natural layout,
            # columns [(bb*NT + n)*C + c].
            if mode == "compute" and g > 0:
                xnat = xnat_pre  # engine-rate probe: reuse group 0's x
            else:
                xnat = xnat_p.tile([128, group * NT * C], F16, tag="xnat")
                xnat_pre = xnat
                bb0 = 0
                while bb0 < group:
                    ch = min(x_chunk, group - bb0)
                    nc.gpsimd.dma_start(
                        xnat[:, bb0 * NT * C:(bb0 + ch) * NT * C],
                        x[g * group + bb0:g * group + bb0 + ch].rearrange(
                            "b (n p) c -> p b n c", p=128),
                    )
                    bb0 += ch
            if mode == "xonly":
                continue
            # ob staging: columns (b, j, h) with token t = 2p + j -> the
            # output DMA sees contiguous 512B (j, h) runs per (p, b).
            ob = ob_p.tile([128, group * NT * H], F32, tag="ob")
            if mode == "dma":
                nc.vector.memset(ob[:], 0.0)
            for pr in range(0 if mode == "dma" else group // 2):
                # qk_ps pair tile (one PSUM bank): batch half's projection
                # at cols half*T:(half+1)*T, partitions 0:64 = qT, 64:128 = kT
                qk_ps = qk_ps_p.tile([128, 2 * T], F32, tag="qk_ps")
                # qk pair SBUF tile [64, (half, q 0:T | k T:2T)]
                qk = qk_p.tile([H, 2 * 2 * T], F16, tag="qk")
                # vx pair tile [128, (half, n, 64 v + 1 one)]
                vx = vx_p.tile([128, 2 * NT * H1], F16, tag="vx")
                # "pe_half" diagnostic: only contraction chunk 0 feeds the
                # projections (wrong outputs; measures PE-rate sensitivity)
                ncc_eff = 1 if mode == "pe_half" else NCC
                # sv tile(s): each batch half's [384 scores | 128 v] is one
                # PSUM bank. pair_sv couples both halves in one 2-bank tile
                # (fewer instructions, but single-buffered: measured slower).
                SB = 512  # f32 cols per half (one bank)
                v_pair = None
                if pair_sv:
                    sv_pair = sv_ps_p.tile([128, 1024], F32, tag="sv_ps")
                    sv_of = [(sv_pair, 0), (sv_pair, SB)]
                elif split_sv:
                    # scores-only tiles: occupancy starts at the scores
                    # matmul instead of the much earlier v projection
                    sv_of = [
                        (sv_ps_p.tile([128, 384], F32, tag="sv_ps",
                                      name="sv_ps"), 0)
                        for _ in range(2)]
                    v_pair = v_ps_p.tile([128, 2 * NT * H], F32, tag="v_ps")
                else:
                    sv_of = [
                        (sv_ps_p.tile([128, 512], F32, tag="sv_ps",
                                      name="sv_ps"), 0)
                        for _ in range(2)]
                # xt_pair: one 2-bank PSUM tile for both batches'
                # transposes + ONE pair-wide DVE copy (individual
                # transposes write 256B-aligned chunks, never crossing
                # the bank boundary at byte 2048)
                if xt_pair:
                    xt_ps2 = xt_ps_p.tile([128, 2 * NCC * T], F16,
                                          tag="xt_ps")
                    xt2 = xt_p.tile([128, 2 * NCC * T], F16, tag="xt")
                for half in range(2):
                    bb = pr * 2 + half
                    sv_t, so = sv_of[half]
                    # --- transpose x -> xT [c, t]; columns [cc*T + t] ---
                    if xt_pair:
                        xt_ps = xt_ps2[:, half * NCC * T:(half + 1) * NCC * T]
                        xt = xt2[:, half * NCC * T:(half + 1) * NCC * T]
                    else:
                        xt_ps = xt_ps_p.tile([128, NCC * T], F16,
                                             tag="xt_ps", name="xt_ps")[:]
                        xt = xt_p.tile([128, NCC * T], F16,
                                       tag="xt", name="xt")[:]
                    for cc in range(ncc_eff):
                        for n in range(NT):
                            nc.tensor.transpose(
                                xt_ps[:, cc * T + n * 128:cc * T + (n + 1) * 128],
                                xnat[:, (bb * NT + n) * C + cc * 128:
                                     (bb * NT + n) * C + (cc + 1) * 128],
                                ident[:],
                            )
                    if not xt_pair:
                        nc.vector.tensor_copy(
                            xt[:, 0:ncc_eff * T], xt_ps[:, 0:ncc_eff * T])

                    projs = []
                    projs.append(("qk", None))
                    projs.append(("v", None))
                    if vfirst:
                        projs.reverse()
                    if xt_pair and half == 0:
                        continue  # projections for both halves after copy
                    halves = [0, 1] if xt_pair else [half]
                    if xt_pair:
                        nc.vector.tensor_copy(xt2[:], xt_ps2[:])
                    for hh in halves:
                        if xt_pair:
                            xt = xt2[:, hh * NCC * T:(hh + 1) * NCC * T]
                            sv_t, so = sv_of[hh]
                        for kind, _ in projs:
                            if kind == "qk":
                                for cc in range(ncc_eff):
                                    st = dict(start=(cc == 0),
                                              stop=(cc == ncc_eff - 1))
                                    nc.tensor.matmul(
                                        qk_ps[:, hh * T:(hh + 1) * T],
                                        wqk_sb[:, cc * 128:(cc + 1) * 128],
                                        xt[:, cc * T:(cc + 1) * T], **st)
                            else:
                                for n in range(NT):
                                    for cc in range(ncc_eff):
                                        st = dict(start=(cc == 0),
                                                  stop=(cc == ncc_eff - 1))
                                        vdst = (
                                            v_pair[:, (hh * NT + n) * H:
                                                   (hh * NT + n + 1) * H]
                                            if split_sv else
                                            sv_t[:, so + 384 + n * H:
                                                 so + 384 + (n + 1) * H])
                                        nc.tensor.matmul(
                                            vdst,
                                            xt[:, cc * T + n * 128:
                                               cc * T + (n + 1) * 128],
                                            wv_sb[:, cc * H:(cc + 1) * H],
                                            **st)

                # --- pair-fused PSUM->SBUF copies on ScalarE ---
                qk4 = qk[:].rearrange("p (b x) -> p b x", x=2 * T)
                if qk_fused:
                    nc.scalar.copy(
                        qk4[:, :, 0:T],
                        qk_ps[0:H, :].rearrange("p (b t) -> p b t", t=T))
                    kt_eng = nc.vector if kt == "dve" else nc.scalar
                    (kt_eng.tensor_copy if kt == "dve" else kt_eng.copy)(
                        qk4[:, :, T:2 * T],
                        qk_ps[H:128, :].rearrange("p (b t) -> p b t", t=T))
                else:
                    for hf in range(2):
                        nc.scalar.copy(qk4[:, hf, 0:T],
                                       qk_ps[0:H, hf * T:(hf + 1) * T])
                        nc.scalar.copy(qk4[:, hf, T:2 * T],
                                       qk_ps[H:128, hf * T:(hf + 1) * T])

                # vx: v copy on DVE (GPSIMD cannot touch PSUM);
                # pair-fused ones memset on GPSIMD (SBUF-only op)
                vx4 = vx[:].rearrange("p (b n x) -> p b n x", n=NT, x=H1)
                if pair_sv:
                    sv2 = sv_of[0][0][:].rearrange("p (b x) -> p b x", x=SB)
                    nc.vector.tensor_copy(
                        vx4[:, :, :, 0:H],
                        sv2[:, :, 384:512].rearrange(
                            "p b (n h) -> p b n h", h=H))
                elif split_sv:
                    nc.vector.tensor_copy(
                        vx4[:, :, :, 0:H],
                        v_pair[:].rearrange("p (b n h) -> p b n h",
                                            n=NT, h=H))
                else:
                    for half in range(2):
                        sv_t, so = sv_of[half]
                        nc.vector.tensor_copy(
                            vx4[:, half, :, 0:H],
                            sv_t[:, so + 384:so + 512].rearrange(
                                "p (n h) -> p n h", h=H))
                if not ones_pre:
                    nc.gpsimd.memset(vx4[:, :, :, H:H1], 1.0)

                # o_ps pair tile (one PSUM bank): batch half at cols
                # half*NT*H1; within a half, o_ps[p, (j, 0:65)], token
                # t = 2p + j, col 64 = softmax denominator.
                o_ps = o_ps_p.tile([128, 2 * NT * H1], F32, tag="o_ps")
                p_list = []
                p_pair = None
                if not pair_sv and pair_mask:
                    # pair SBUF tile: per-batch exp writes halves; ONE
                    # mask instruction covers both batches' diag blocks
                    p_pair = p_p.tile([128, 2 * (T + 128)], F16, tag="p_sb")
                for half in range(2):
                    sv_t, so = sv_of[half]
                    q0 = half * 2 * T           # qT cols for this batch
                    k0 = half * 2 * T + T       # kT cols
                    # --- scores (transposed): S'[s, t] = kT.T @ qT ---
                    # S'0: s in [0,128), all t (cols 0:256)
                    # S'1: s in [128,256), t in [128,256) (cols 256:384)
                    nc.tensor.matmul(
                        sv_t[:, so:so + T], qk[:, k0:k0 + 128],
                        qk[:, q0:q0 + T])
                    nc.tensor.matmul(
                        sv_t[:, so + T:so + T + 128],
                        qk[:, k0 + 128:k0 + T], qk[:, q0 + 128:q0 + T])

                    if not pair_sv:
                        # --- per-batch exp ---
                        if pair_mask:
                            p_sb = p_pair[:, half * (T + 128):
                                          (half + 1) * (T + 128)]
                        else:
                            p_sb = p_p.tile([128, T + 128], F16,
                                            tag="p_sb", name="p_sb")[:]
                        p_list.append(p_sb)
                        nc.scalar.activation(p_sb, sv_t[:, so:so + T + 128],
                                             AF.Exp, scale=SCALE)
                        if not pair_mask:
                            p3 = p_sb.rearrange(
                                "p (k x) -> p k x", x=128)[:, 0:3:2, :]
                            if mask == "pool":
                                nc.gpsimd.affine_select(
                                    p3, p3, pattern=[[0, 2], [1, 128]],
                                    compare_op=ALU.is_ge, fill=0.0, base=0,
                                    channel_multiplier=-1,
                                )
                            elif mask == "dve":
                                tri2 = tri[:].rearrange(
                                    "p (k x) -> p k x", k=1).broadcast_to(
                                    [128, 2, 128])
                                nc.vector.tensor_mul(p3, p3, tri2)
                if not pair_sv and pair_mask:
                    p3 = p_pair[:].rearrange(
                        "p (b k x) -> p b k x", x=128, k=3)[:, :, 0:3:2, :]
                    if mask == "pool":
                        nc.gpsimd.affine_select(
                            p3, p3, pattern=[[0, 2], [0, 2], [1, 128]],
                            compare_op=ALU.is_ge, fill=0.0, base=0,
                            channel_multiplier=-1,
                        )
                    elif mask == "dve":
                        tri2 = tri[:].rearrange(
                            "p (b k x) -> p b k x", b=1, k=1).broadcast_to(
                            [128, 2, 2, 128])
                        nc.vector.tensor_mul(p3, p3, tri2)

                if pair_sv:
                    # --- pair-fused exp (scale folded in) on ScalarE ---
                    p_pair = p_p.tile([128, 2 * (T + 128)], F16, tag="p_sb")
                    p_list = [p_pair[:, 0:T + 128], p_pair[:, T + 128:]]
                    p2 = p_pair[:].rearrange("p (b x) -> p b x", x=T + 128)
                    nc.scalar.activation(p2, sv2[:, :, 0:T + 128], AF.Exp,
                                         scale=SCALE)
                    # --- pair-fused causal mask over the diagonal blocks ---
                    p3 = p_pair[:].rearrange(
                        "p (b k x) -> p b k x", x=128, k=3)[:, :, 0:3:2, :]
                    if mask == "pool":
                        nc.gpsimd.affine_select(
                            p3, p3, pattern=[[0, 2], [0, 2], [1, 128]],
                            compare_op=ALU.is_ge, fill=0.0, base=0,
                            channel_multiplier=-1,
                        )
                    elif mask == "dve":
                        tri2 = tri[:].rearrange(
                            "p (b k x) -> p b k x", b=1, k=1).broadcast_to(
                            [128, 2, 2, 128])
                        nc.vector.tensor_mul(p3, p3, tri2)

                for half in range(2):
                    # --- out in pair layout via stride-2 stationaries ---
                    # t < 128 (parts 0:64): only s-chunk 0 contributes;
                    # t >= 128 (parts 64:128): both s-chunks accumulate.
                    # P' columns viewed as (t, parity): block0 = S'0 t<128,
                    # block1 = S'0 t>=128, block2 = S'1 (t>=128, s-chunk 1)
                    pj = p_list[half].rearrange(
                        "p (t two) -> p two t", two=2)
                    o0 = half * NT * H1
                    for j in range(2):
                        vxh = vx4[:, half]
                        if out4:
                            # mm1 covers all 128 out partitions (s-chunk 0);
                            # mm2 accumulates s-chunk 1 onto parts 64:128
                            # only. start/stop describe sim accumulation
                            # groups, not hardware: the split-partition
                            # continuation is HW-correct.
                            nc.tensor.matmul(
                                o_ps[:, o0 + j * H1:o0 + (j + 1) * H1],
                                pj[:, j, 0:128], vxh[:, 0, :],
                                start=True, stop=True, skip_group_check=True)
                            nc.tensor.matmul(
                                o_ps[H:128, o0 + j * H1:o0 + (j + 1) * H1],
                                pj[:, j, 128:192], vxh[:, 1, :],
                                start=False, stop=True, skip_group_check=True)
                        else:
                            nc.tensor.matmul(
                                o_ps[0:H, o0 + j * H1:o0 + (j + 1) * H1],
                                pj[:, j, 0:64], vxh[:, 0, :])
                            nc.tensor.matmul(
                                o_ps[H:128, o0 + j * H1:o0 + (j + 1) * H1],
                                pj[:, j, 64:128], vxh[:, 0, :],
                                start=True, stop=False)
                            nc.tensor.matmul(
                                o_ps[H:128, o0 + j * H1:o0 + (j + 1) * H1],
                                pj[:, j, 128:192], vxh[:, 1, :],
                                start=False, stop=True)

                # --- pair-fused normalization on DVE into staging ---
                o3 = o_ps[:].rearrange("p (q x) -> p q x", x=H1)  # q=(b,j)
                ob_v = ob[:, pr * 2 * NT * H:(pr + 1) * 2 * NT * H].rearrange(
                    "p (q h) -> p q h", h=H)
                if norm == "div":
                    # single fused instruction: o / denominator
                    nc.vector.tensor_tensor(
                        ob_v, o3[:, :, 0:H],
                        o3[:, :, H:H1].broadcast_to([128, 2 * NT, H]),
                        op=ALU.divide)
                else:
                    rec = r_p.tile([128, 2 * NT], F32, tag="rec")
                    nc.vector.reciprocal(rec[:], o3[:, :, H])
                    nc.vector.tensor_mul(
                        ob_v, o3[:, :, 0:H],
                        rec[:].rearrange("p (q o) -> p q o", o=1).broadcast_to(
                            [128, 2 * NT, H]))

            # two 256KB HWDGE DMAs per group; DRAM runs are 512B (j, h)
            # pairs thanks to the pair layout.
            half_g = group // 2
            for hh in range(2):
                nc.sync.dma_start(
                    out[g * group + hh * half_g:
                        g * group + (hh + 1) * half_g].rearrange(
                        "b (p j) h -> p b j h", j=NT),
                    ob[:, hh * half_g * NT * H:(hh + 1) * half_g * NT * H]
                    .rearrange("p (b j h) -> p b j h", j=NT, h=H))
        if loop_cm is not None:
            loop_cm.__exit__(None, None, None)

    nc.compile()
    return nc


_CACHED = {}


def _make_runner(nc):
    """Build a cached shard_map'd jit for an SPMD Bass program."""
    import jax
    from jax.experimental.shard_map import shard_map
    from jax.sharding import Mesh, NamedSharding, PartitionSpec

    import concourse.mybir as mybir
    from concourse.bass2jax import (
        _bass_exec_p, install_neuronx_cc_hook, partition_id_tensor)

    install_neuronx_cc_hook()

    partition_name = (
        nc.partition_id_tensor.name if nc.partition_id_tensor else None)
    in_names, out_names, out_avals, zero_outs = [], [], [], []
    for alloc in nc.m.functions[0].allocations:
        if not isinstance(alloc, mybir.MemoryLocationSet):
            continue
        name = alloc.memorylocations[0].name
        if alloc.kind == "ExternalInput":
            if name != partition_name:
                in_names.append(name)
        elif alloc.kind == "ExternalOutput":
            out_names.append(name)
            shape = tuple(alloc.tensor_shape)
            dtype = mybir.dt.np(alloc.dtype)
            out_avals.append(jax.core.ShapedArray(shape, dtype))
            zero_outs.append(np.zeros(shape, dtype))
    n_params = len(in_names)
    all_in = in_names + out_names
    if partition_name is not None:
        all_in = all_in + [partition_name]

    def _body(*args):
        operands = list(args)
        if partition_name is not None:
            operands.append(partition_id_tensor())
        outs = _bass_exec_p.bind(
            *operands,
            out_avals=tuple(out_avals),
            in_names=tuple(all_in),
            out_names=tuple(out_names),
            lowering_input_output_aliases=(),
            sim_require_finite=False,
            sim_require_nnan=False,
            nc=nc,
        )
        return tuple(outs)

    devices = jax.devices()[:N_CORES]
    mesh = Mesh(np.asarray(devices), ("core",))
    spec = PartitionSpec("core")
    n_args = n_params + len(out_names)
    sharded = jax.jit(
        shard_map(
            _body, mesh=mesh, in_specs=(spec,) * n_args,
            out_specs=(spec,) * len(out_names), check_rep=False,
        ),
        keep_unused=True,
    )
    sharding = NamedSharding(mesh, spec)
    return sharded, in_names, zero_outs, sharding


def _get_runner():
    if "runner" not in _CACHED:
        _CACHED["runner"] = _make_runner(build_nc())
    return _CACHED["runner"]


def _device_inputs(x, Wq, Wk, Wv, runner=None):
    import jax

    sharded, in_names, zero_outs, sharding = runner or _get_runner()
    x = np.ascontiguousarray(x, dtype=np.float32)
    assert x.shape == (B, T, C)
    host = {
        "x": x,
        "Wq": np.concatenate([np.asarray(Wq, np.float32)] * N_CORES, axis=0),
        "Wk": np.concatenate([np.asarray(Wk, np.float32)] * N_CORES, axis=0),
        "Wv": np.concatenate([np.asarray(Wv, np.float32)] * N_CORES, axis=0),
    }
    args = [host[n] for n in in_names]
    args += [
        np.zeros((N_CORES * z.shape[0], *z.shape[1:]), z.dtype) for z in zero_outs
    ]
    return [jax.device_put(a, sharding) for a in args]


def kernel(x, Wq, Wk, Wv):
    sharded, _, _, _ = _get_runner()
    args = _device_inputs(x, Wq, Wk, Wv)
    (out,) = sharded(*args)
    return np.asarray(out)
